# revision 2
# baseline (speedup 1.0000x reference)
"""GAT (4-layer, 2-head) message-passing kernel for 8 TRN2 NeuronCores, v2.

(the f16 alias below is float16, not bfloat16 - fp16 mantissa needed for 2e-2 gate)

Same dst-sharded, degree-sorted chunk layout as v1, restructured for speed:
  - Gather table in BF16, rows [xl(128)|asrc(2)] = 260B (was 544B fp32):
    halves indirect-gather and AllGather bytes.
  - h and r kept resident in SBUF across layers; the dense phase does
    transpose+matmul per chunk with no DMA loads/stores; r goes to DRAM in
    ONE strided DMA per layer (for the AllGather).
  - Denominator computed by reducing the weights directly (no ones-columns
    in the table); pad slots hit a poison row (asrc=-60 => w~0).
  - Weighted message reduce done as contiguous pairwise tree-adds over k
    (the baseline's strided k-reduce paid the >8B-stride DVE penalty).
  - Deeper gather prefetch (4 chunk buffers).
"""

import math
import os
import numpy as np

# ---------------------------------------------------------------- problem dims
N_NODES = 100000
N_EDGES = 1600000
N_CORES = 8
DIM_IN = 128
HEADS = 2
HID = 64
DIM_OUT = 32

PART = 128        # nodes per chunk / SBUF partitions


class Cfg:
    def __init__(self, n_nodes=N_NODES, n_edges=N_EDGES, n_cores=N_CORES):
        assert n_nodes % n_cores == 0
        self.n_nodes = n_nodes
        self.n_edges = n_edges
        self.n_cores = n_cores
        self.shard = n_nodes // n_cores
        self.nchunk = math.ceil(self.shard / PART)
        self.nrow = self.nchunk * PART + 1      # +1 pad/poison row
        self.nrows_all = self.nrow * n_cores
        self.caps = None
        # per-layer widths: XL = transformed feature width (both heads),
        # GW = gathered row width (XL + 2 asrc), RW = r width (GW + 2 adst)
        self.XL = [128, 128, 128, 64]
        self.GW = [130, 130, 130, 66]
        self.RW = [132, 132, 132, 68]
        self.OW = [128, 128, 128, 32]   # out width per layer

    def finalize_caps(self, caps):
        self.caps = [int(c) for c in caps]
        self.nblk = sum(self.caps)
        self.blk_start = np.concatenate([[0], np.cumsum(self.caps)]).astype(np.int64)


# ------------------------------------------------------------------ host prep
def preprocess(cfg: Cfg, edge_index: np.ndarray):
    src = np.asarray(edge_index[0], dtype=np.int64)
    dst = np.asarray(edge_index[1], dtype=np.int64)
    C, SH, NC = cfg.n_cores, cfg.shard, cfg.nchunk

    owner = dst // SH
    deg = np.zeros((C, SH), dtype=np.int64)
    for c in range(C):
        deg[c] = np.bincount(dst[owner == c] - c * SH, minlength=SH)

    # per-core: sort nodes by degree desc -> chunk j = nodes[j*128:(j+1)*128]
    slot_node = np.full((C, cfg.nrow), -1, dtype=np.int64)
    node_slot = np.full(cfg.n_nodes, -1, dtype=np.int64)   # slot within core
    maxdeg = np.zeros((C, NC), dtype=np.int64)
    for c in range(C):
        order = np.argsort(-deg[c], kind="stable")
        ns = min(SH, NC * PART)
        slot_node[c, :ns] = order[:ns] + c * SH
        node_slot[order + c * SH] = np.arange(SH)
        d_sorted = deg[c][order]
        pad = np.zeros(NC * PART - SH, dtype=np.int64)
        d_pad = np.concatenate([d_sorted, pad])
        maxdeg[c] = d_pad.reshape(NC, PART).max(axis=1)
    caps = np.maximum(maxdeg.max(axis=0), 1)
    cfg.finalize_caps(caps)

    # global table row of node n (within its core's region)
    node_row = np.full(cfg.n_nodes, -1, dtype=np.int64)
    for c in range(C):
        rows = slot_node[c]
        real = rows >= 0
        node_row[rows[real]] = c * cfg.nrow + np.nonzero(real)[0]
    assert (node_row >= 0).all()
    pad_row = np.array([c * cfg.nrow + cfg.nrow - 1 for c in range(C)])

    # place edges: edge -> (core, slot=node_slot[dst], k=arrival index)
    NBLK = cfg.nblk
    srcg = np.zeros((C, PART, NBLK), dtype=np.int64)
    for c in range(C):
        srcg[c, :, :] = pad_row[c]
    ds_order = np.argsort(dst, kind="stable")
    ds = dst[ds_order]
    uniq, first = np.unique(ds, return_index=True)
    cnt = np.diff(np.concatenate([first, [len(ds)]]))
    k_of = np.arange(len(ds)) - np.repeat(first, cnt)   # arrival index per dst
    e_sl = node_slot[ds]
    e_ch = e_sl // PART
    e_pa = e_sl % PART
    e_ow = ds // SH
    col = cfg.blk_start[e_ch] + k_of
    srcg[e_ow, e_pa, col] = node_row[src[ds_order]]

    return dict(srcg=srcg.astype(np.int32), slot_node=slot_node,
                node_row=node_row)


def pack_weights(cfg, W, asrc, adst, layer):
    """Wcat [128, RW]: [W_h0 | W_h1 | W@As(2) | W@Ad(2)]."""
    XL, RW = cfg.XL[layer], cfg.RW[layer]
    dout = XL // 2
    Wcat = np.zeros((W.shape[0], RW), dtype=np.float32)
    for h in range(2):
        Wcat[:, h * dout:(h + 1) * dout] = W[:, h * dout:(h + 1) * dout]
        Wcat[:, XL + h] = W[:, h * dout:(h + 1) * dout] @ asrc[h]
        Wcat[:, XL + 2 + h] = W[:, h * dout:(h + 1) * dout] @ adst[h]
    return Wcat


# ---------------------------------------------------------------- numpy model
def emulate_numpy(cfg, prep, inputs):
    """fp32 emulation of the device algorithm (no bf16 rounding)."""
    C = cfg.n_cores
    x = np.asarray(inputs["x"], np.float32)
    params = [(pack_weights(cfg, np.asarray(inputs[f"W{l}"], np.float32),
                            np.asarray(inputs[f"asrc{l}"], np.float32),
                            np.asarray(inputs[f"adst{l}"], np.float32), l),
               np.asarray(inputs[f"b{l}"], np.float32)) for l in range(4)]
    srcg = prep["srcg"]
    slot_node = prep["slot_node"]

    h = np.zeros((C, cfg.nrow - 1, DIM_IN), np.float32)
    for c in range(C):
        sn = slot_node[c][:-1]
        real = sn >= 0
        h[c][real] = x[sn[real]]

    out_final = np.zeros((C, cfg.nrow - 1, DIM_OUT), np.float32)
    for l in range(4):
        Wcat, b = params[l]
        XL, GW, RW = cfg.XL[l], cfg.GW[l], cfg.RW[l]
        r = np.einsum("cnk,kr->cnr", h, Wcat)          # [C, nrow-1, RW]
        # gathered table: [C*nrow, GW] with poison pad row per core
        tab = np.zeros((C * cfg.nrow, GW), np.float32)
        for c in range(C):
            tab[c * cfg.nrow:c * cfg.nrow + cfg.nrow - 1] = r[c][:, :GW]
            tab[c * cfg.nrow + cfg.nrow - 1, XL:XL + 2] = -60.0
        for c in range(C):
            for j in range(cfg.nchunk):
                K = cfg.caps[j]
                cols = slice(cfg.blk_start[j], cfg.blk_start[j] + K)
                G = tab[srcg[c][:, cols]]               # [128, K, GW]
                ad = r[c, j * PART:(j + 1) * PART, XL + 2:XL + 4]
                z = G[:, :, XL:XL + 2] + ad[:, None, :]
                zlr = np.where(z > 0, z, 0.2 * z)
                w = np.exp(zlr)                         # [128, K, 2]
                den = w.sum(axis=1) + 1e-16             # [128, 2]
                prod = G[:, :, :XL].reshape(PART, K, 2, XL // 2) \
                    * w[:, :, :, None]
                nm = prod.sum(axis=1)                   # [128, 2, XL//2]
                o = nm / den[:, :, None]
                if l < 3:
                    hn = o.reshape(PART, XL) + b[None, :]
                    h[c, j * PART:(j + 1) * PART] = np.maximum(hn, 0.0)
                else:
                    m = o.mean(axis=1) + b[None, :]
                    mx = m.max(axis=1, keepdims=True)
                    lse = mx + np.log(np.exp(m - mx).sum(axis=1, keepdims=True))
                    out_final[c, j * PART:(j + 1) * PART] = m - lse
    res = np.zeros((cfg.n_nodes, DIM_OUT), np.float32)
    for c in range(C):
        sn = slot_node[c][:-1]
        real = sn >= 0
        res[sn[real]] = out_final[c][real]
    return res


# ---------------------------------------------------------------- bass kernel
def build_nc(cfg: Cfg):
    import concourse.bass as bass
    import concourse.mybir as mybir
    import concourse.tile as tile
    from concourse import bacc
    from concourse.masks import make_identity

    f32 = mybir.dt.float32
    bf16 = mybir.dt.float16
    i32 = mybir.dt.int32
    AF = mybir.ActivationFunctionType
    OP = mybir.AluOpType

    nc = bacc.Bacc(num_devices=cfg.n_cores)
    NBLK, NCH = cfg.nblk, cfg.nchunk

    x_in = nc.declare_dram_parameter("x", [PART, NCH * PART], f32, False)
    srcg_in = nc.declare_dram_parameter("srcg", [PART, NBLK], i32, False)
    wcat_in = [nc.declare_dram_parameter(f"wcat{l}", [DIM_IN, cfg.RW[l]], f32, False)
               for l in range(4)]
    bias_in = [nc.declare_dram_parameter(f"bias{l}", [PART, cfg.OW[l]], f32, False)
               for l in range(4)]
    poison_in = nc.declare_dram_parameter("poison", [1, 130], bf16, False)
    poison3_in = nc.declare_dram_parameter("poison3", [1, 66], bf16, False)
    out_p = nc.declare_dram_parameter("out", [NCH * PART, DIM_OUT], f32, True)

    rshA = nc.dram_tensor("rshA", [cfg.nrow, 130], bf16)
    rshB = nc.dram_tensor("rshB", [cfg.nrow, 130], bf16)
    rsh3 = nc.dram_tensor("rsh3", [cfg.nrow, 66], bf16)
    aspc = "Shared" if cfg.n_cores > 4 else "Local"
    rfA = nc.dram_tensor("rfA", [cfg.nrows_all, 130], bf16, addr_space=aspc)
    rfB = nc.dram_tensor("rfB", [cfg.nrows_all, 130], bf16, addr_space=aspc)
    rf3 = nc.dram_tensor("rf3", [cfg.nrows_all, 66], bf16, addr_space=aspc)

    rg = [list(range(cfg.n_cores))]

    with tile.TileContext(nc) as tc:
        with tc.tile_pool(name="const", bufs=1) as cp, \
             tc.tile_pool(name="state", bufs=1) as sp, \
             tc.tile_pool(name="gbuf", bufs=4) as gp, \
             tc.tile_pool(name="prod", bufs=2) as prp, \
             tc.tile_pool(name="small", bufs=3) as mp, \
             tc.tile_pool(name="dense", bufs=3) as dp, \
             tc.tile_pool(name="psd", bufs=2, space="PSUM") as ppd:

            srcg = cp.tile([PART, NBLK], i32)
            nc.sync.dma_start(out=srcg[:], in_=srcg_in[:])
            wct, bst = [], []
            for l in range(4):
                t = cp.tile([DIM_IN, cfg.RW[l]], f32, tag=f"wc{l}")
                nc.sync.dma_start(out=t[:], in_=wcat_in[l][:])
                wct.append(t)
                t2 = cp.tile([PART, cfg.OW[l]], f32, tag=f"bs{l}")
                nc.sync.dma_start(out=t2[:], in_=bias_in[l][:])
                bst.append(t2)
            ident = cp.tile([PART, PART], f32)
            make_identity(nc, ident[:])

            # resident state: h [node-part, chunk*feat], r (bf16, incl adst),
            # fin (final output staging)
            h_sb = sp.tile([PART, NCH * DIM_IN], f32, tag="h")
            nc.sync.dma_start(out=h_sb[:], in_=x_in[:])
            r_sb = sp.tile([PART, NCH * 132], bf16, tag="r")
            fin_sb = sp.tile([PART, NCH * DIM_OUT], f32, tag="fin")

            for l in range(4):
                XL, GW, RW = cfg.XL[l], cfg.GW[l], cfg.RW[l]
                rsh = (rshA, rshB, rshA, rsh3)[l]
                rfull = (rfA, rfB, rfA, rf3)[l]
                rv = r_sb[:].rearrange("p (j r) -> p j r", r=132)
                hv = h_sb[:].rearrange("p (j f) -> p j f", f=DIM_IN)

                # ---------------- dense phase ----------------
                for g in range(NCH):
                    pt = ppd.tile([PART, PART], f32, tag="ht")
                    nc.tensor.transpose(out=pt[:], in_=hv[:, g, :],
                                        identity=ident[:])
                    hts = dp.tile([PART, PART], f32, tag="hts")
                    nc.vector.tensor_copy(out=hts[:], in_=pt[:])
                    pr = ppd.tile([PART, RW], f32, tag="pr")
                    nc.tensor.matmul(out=pr[:], lhsT=hts[:], rhs=wct[l][:],
                                     start=True, stop=True)
                    nc.vector.tensor_copy(out=rv[:, g, 0:RW], in_=pr[:])

                # one strided store of the gather payload [nrow-1, GW]
                nc.sync.dma_start(
                    out=rsh[0:NCH * PART, :]
                        .rearrange("(j p) r -> p j r", p=PART),
                    in_=rv[:, :, 0:GW])
                pin = poison_in if l < 3 else poison3_in
                nc.sync.dma_start(out=rsh[cfg.nrow - 1:cfg.nrow, 0:GW],
                                  in_=pin[0:1, 0:GW])

                # ---------------- all-gather ----------------
                nc.gpsimd.collective_compute(
                    "AllGather", mybir.AluOpType.bypass, replica_groups=rg,
                    ins=[rsh[:]], outs=[rfull[:]])

                # ---------------- edge phase ----------------
                for g in range(NCH):
                    K = cfg.caps[g]
                    b0 = int(cfg.blk_start[g])

                    gt = gp.tile([PART, K * GW], bf16, tag="g")
                    gv = gt[:].rearrange("p (k r) -> p k r", r=GW)
                    for k in range(K):
                        nc.gpsimd.indirect_dma_start(
                            out=gv[:, k, :], out_offset=None, in_=rfull[:],
                            in_offset=bass.IndirectOffsetOnAxis(
                                ap=srcg[:, b0 + k:b0 + k + 1], axis=0))

                    # z = asrc[src] + adst[dst-broadcast]; w = exp(lrelu(z))
                    zt = mp.tile([PART, K * 2], f32, tag="z")
                    nc.vector.tensor_tensor(
                        out=zt[:].rearrange("p (k t) -> p k t", t=2),
                        in0=gv[:, :, XL:XL + 2],
                        in1=rv[:, g, RW - 2:RW].unsqueeze(1)
                            .to_broadcast([PART, K, 2]),
                        op=OP.add)
                    lt = mp.tile([PART, K * 2], f32, tag="lr")
                    nc.vector.tensor_scalar_mul(out=lt[:], in0=zt[:],
                                                scalar1=0.2)
                    nc.vector.tensor_tensor(out=lt[:], in0=lt[:], in1=zt[:],
                                            op=OP.max)
                    wt = mp.tile([PART, K * 2], bf16, tag="w")
                    nc.scalar.activation(out=wt[:], in_=lt[:], func=AF.Exp)

                    # weighted messages, fp32 [p, k, XL]
                    pd = prp.tile([PART, K * XL], f32, tag="pd")
                    nc.vector.tensor_tensor(
                        out=pd[:].rearrange("p (k h c) -> p k h c", h=2,
                                            c=XL // 2),
                        in0=gv[:, :, 0:XL].rearrange("p k (h c) -> p k h c",
                                                     c=XL // 2),
                        in1=wt[:].rearrange("p (k h) -> p k h", h=2)
                            .unsqueeze(3).to_broadcast([PART, K, 2, XL // 2]),
                        op=OP.mult)

                    # contiguous pairwise tree-reduce over k
                    pv = pd[:].rearrange("p (k c) -> p k c", c=XL)
                    n = K
                    while n > 1:
                        m = n // 2
                        nc.vector.tensor_tensor(
                            out=pv[:, 0:m, :], in0=pv[:, 0:m, :],
                            in1=pv[:, m:2 * m, :], op=OP.add)
                        if n % 2:
                            nc.vector.tensor_tensor(
                                out=pv[:, 0, :], in0=pv[:, 0, :],
                                in1=pv[:, 2 * m, :], op=OP.add)
                        n = m
                    # nm = pv[:, 0, :]  ([p, XL])

                    # denominator: reduce w over k -> [p, 2]
                    dt2 = mp.tile([PART, 2], f32, tag="den")
                    nc.vector.tensor_reduce(
                        out=dt2[:],
                        in_=wt[:].rearrange("p (k t) -> p t k", t=2),
                        axis=mybir.AxisListType.X, op=OP.add)
                    nc.vector.tensor_scalar(out=dt2[:], in0=dt2[:],
                                            scalar1=1e-16, scalar2=None,
                                            op0=OP.add)
                    rc = mp.tile([PART, 2], f32, tag="rcp")
                    nc.vector.reciprocal(out=rc[:], in_=dt2[:])

                    if l < 3:
                        ot = dp.tile([PART, XL], f32, tag="o")
                        nc.vector.tensor_tensor(
                            out=ot[:].rearrange("p (h c) -> p h c",
                                                c=XL // 2),
                            in0=pv[:, 0, :].rearrange("p (h c) -> p h c",
                                                      c=XL // 2),
                            in1=rc[:].unsqueeze(2)
                                .to_broadcast([PART, 2, XL // 2]),
                            op=OP.mult)
                        nc.vector.tensor_tensor(out=ot[:], in0=ot[:],
                                                in1=bst[l][:], op=OP.add)
                        nc.vector.tensor_scalar_max(out=hv[:, g, :],
                                                    in0=ot[:], scalar1=0.0)
                    else:
                        oh = mp.tile([PART, XL], f32, tag="oh")
                        nc.vector.tensor_tensor(
                            out=oh[:].rearrange("p (h c) -> p h c",
                                                c=XL // 2),
                            in0=pv[:, 0, :].rearrange("p (h c) -> p h c",
                                                      c=XL // 2),
                            in1=rc[:].unsqueeze(2)
                                .to_broadcast([PART, 2, XL // 2]),
                            op=OP.mult)
                        m1 = mp.tile([PART, XL // 2], f32, tag="m1")
                        ohv = oh[:].rearrange("p (h c) -> p h c", c=XL // 2)
                        nc.vector.tensor_tensor(out=m1[:], in0=ohv[:, 0, :],
                                                in1=ohv[:, 1, :], op=OP.add)
                        nc.vector.tensor_scalar_mul(out=m1[:], in0=m1[:],
                                                    scalar1=0.5)
                        nc.vector.tensor_tensor(out=m1[:], in0=m1[:],
                                                in1=bst[l][:], op=OP.add)
                        mx = mp.tile([PART, 1], f32, tag="mx")
                        nc.vector.tensor_reduce(out=mx[:], in_=m1[:],
                                                axis=mybir.AxisListType.X,
                                                op=OP.max)
                        sh = mp.tile([PART, XL // 2], f32, tag="sh")
                        nc.vector.tensor_scalar(out=sh[:], in0=m1[:],
                                                scalar1=mx[:], scalar2=None,
                                                op0=OP.subtract)
                        ex = mp.tile([PART, XL // 2], f32, tag="ex")
                        nc.scalar.activation(out=ex[:], in_=sh[:], func=AF.Exp)
                        sm = mp.tile([PART, 1], f32, tag="sm")
                        nc.vector.tensor_reduce(out=sm[:], in_=ex[:],
                                                axis=mybir.AxisListType.X,
                                                op=OP.add)
                        ln = mp.tile([PART, 1], f32, tag="ln")
                        nc.scalar.activation(out=ln[:], in_=sm[:], func=AF.Ln)
                        fv = fin_sb[:].rearrange("p (j r) -> p j r",
                                                 r=DIM_OUT)
                        nc.vector.tensor_scalar(out=fv[:, g, :], in0=sh[:],
                                                scalar1=ln[:], scalar2=None,
                                                op0=OP.subtract)

            nc.sync.dma_start(
                out=out_p[:].rearrange("(j p) r -> p j r", p=PART),
                in_=fin_sb[:].rearrange("p (j r) -> p j r", r=DIM_OUT))
    return nc


def make_in_maps(cfg, prep, inputs):
    x = np.asarray(inputs["x"], np.float32)
    in_maps = []
    for c in range(cfg.n_cores):
        sn = prep["slot_node"][c]
        xs = np.zeros((PART, cfg.nchunk * DIM_IN), np.float32)
        for j in range(cfg.nchunk):
            sl = sn[j * PART:(j + 1) * PART]
            real = sl >= 0
            blk = np.zeros((PART, DIM_IN), np.float32)
            blk[real] = x[sl[real]]
            xs[:, j * DIM_IN:(j + 1) * DIM_IN] = blk
        import ml_dtypes
        poison = np.zeros((1, 130), np.float16)
        poison[0, 128:130] = -60.0
        poison3 = np.zeros((1, 66), np.float16)
        poison3[0, 64:66] = -60.0
        m = dict(x=xs, srcg=prep["srcg"][c], poison=poison, poison3=poison3)
        for l in range(4):
            m[f"wcat{l}"] = pack_weights(
                cfg, np.asarray(inputs[f"W{l}"], np.float32),
                np.asarray(inputs[f"asrc{l}"], np.float32),
                np.asarray(inputs[f"adst{l}"], np.float32), l)
            b = np.asarray(inputs[f"b{l}"], np.float32)
            m[f"bias{l}"] = np.broadcast_to(b[None, :], (PART, cfg.OW[l])).copy()
        in_maps.append(m)
    return in_maps


# ---------------------------------------------------------------- entry point
def kernel(**inputs) -> np.ndarray:
    cfg = Cfg()
    edge_index = np.asarray(inputs["edge_index"])
    prep = preprocess(cfg, edge_index)
    nc = build_nc(cfg)
    nc.finalize()
    in_maps = make_in_maps(cfg, prep, inputs)

    from concourse.bass_utils import run_bass_kernel_spmd
    res = run_bass_kernel_spmd(nc, in_maps, list(range(cfg.n_cores)),
                               trace=bool(int(os.environ.get("GAT_TRACE", "0"))))
    if res.exec_time_ns is not None:
        print(f"HW exec time: {res.exec_time_ns} ns")
    out = np.zeros((cfg.n_nodes, DIM_OUT), np.float32)
    for c in range(cfg.n_cores):
        sn = prep["slot_node"][c][:-1]
        real = sn >= 0
        out[sn[real]] = res.results[c]["out"][real]
    return out
